# revision 49
# baseline (speedup 1.0000x reference)
"""AttentionBlock kernel for 8 Trainium2 NeuronCores.

Reference op: GroupNorm(8 groups) -> 1x1 conv qkv -> 8-head attention over
1024 spatial positions -> 1x1 conv proj -> residual.   Shapes (full):
x [8, 512, 32, 32]; qkv_w [1536, 512]; proj_w [512, 512].

Sharding: pure data-parallel over batch - one batch element per core.

Per-core design (v2, exp-stream pipelined):
  - x loaded in bf16 (halves input DMA; residual quantization ~4e-3 abs).
  - q/k biases folded into the PSUM->SBUF copies (DVE tensor_scalar add);
    the 1/sqrt(hd) attention scale is folded into the k weights+bias on
    host.  v bias is folded into the proj bias on host (softmax weights
    sum to 1, so  proj_w @ vb  is an exact per-channel constant).
  - Scores computed transposed (S^T = K^T Q) per (pair, key-chunk, head):
    two matmuls (head even / head odd) occupy row-tiles (0,0)/(64,0) of
    the PE array and run concurrently.
  - exp on ACT engine at FD=1024 per tile (the engine floor: 64 tiles x
    ~1.15us ~= 74us governs the kernel); PSUM scores double-buffered so
    the ACT stream never stalls.
  - AV rides the ones-column trick (65th row = softmax denominator),
    accumulated per (hh, nj) in batched matmul groups.
  - Softmax normalization: AV PSUM is released by cheap DVE copies (data
    rows -> ou_sb, sum row -> a staging row that is DMA-gathered into a
    [4,N] tile).  The reciprocal then runs OFF the critical path: one DVE
    reciprocal [4,N] covering pairs 0-1 mid-stream, and an ACT ln/exp
    chain for pairs 2-3 at the tail (ACT is idle once the exp stream
    ends).  Broadcast back via DRAM bounce, final o = ou * rec on DVE.
"""

import os

import numpy as np
import ml_dtypes

NCORES = 8
C = 512
N = 1024  # 32*32 spatial
NH = 8
HD = 64  # head dim
CCH = 4  # channel chunks of 128
EPS = 1e-5

_CACHE = {}
LAST = {"exec_time_ns": None, "results": None}


def _build_program():
    import concourse.bass as bass
    import concourse.tile as tile
    from concourse import mybir

    f32 = mybir.dt.float32
    bf16 = mybir.dt.bfloat16
    AF = mybir.ActivationFunctionType
    OP = mybir.AluOpType

    nc = bass.Bass()

    # ---- DRAM parameters (per core). Host pre-reshapes/pre-transposes. ----
    x_d = nc.declare_dram_parameter("x", [128, CCH, N], bf16, isOutput=False)
    qkvw_d = nc.declare_dram_parameter("qkv_wT", [128, CCH, 3 * C], bf16, isOutput=False)
    qb_d = nc.declare_dram_parameter("qb", [128, CCH], f32, isOutput=False)
    kb_d = nc.declare_dram_parameter("kb", [128, CCH], f32, isOutput=False)
    pw_d = nc.declare_dram_parameter("proj_wT", [128, CCH, C], bf16, isOutput=False)
    pb_d = nc.declare_dram_parameter("pb", [CCH, 128, 1], f32, isOutput=False)
    gnw_d = nc.declare_dram_parameter("gnw", [128, CCH], f32, isOutput=False)
    gnb_d = nc.declare_dram_parameter("gnb", [128, CCH], f32, isOutput=False)
    mask_d = nc.declare_dram_parameter("gn_mask", [128, 128], f32, isOutput=False)
    id_d = nc.declare_dram_parameter("id128", [128, 128], bf16, isOutput=False)
    out_d = nc.declare_dram_parameter("out", [CCH, 128, N], f32, isOutput=True)

    from contextlib import ExitStack

    with (
        nc.allow_low_precision(reason="bf16 tiles feed full-speed matmuls"),
        tile.TileContext(nc) as tc,
        ExitStack() as ctx,
    ):
        consts = ctx.enter_context(tc.tile_pool(name="consts", bufs=1))
        xp = ctx.enter_context(tc.tile_pool(name="xp", bufs=1))
        wqp = ctx.enter_context(tc.tile_pool(name="wqp", bufs=1))
        xnp = ctx.enter_context(tc.tile_pool(name="xnp", bufs=1))
        qkp = ctx.enter_context(tc.tile_pool(name="qkp", bufs=1))
        vtp = ctx.enter_context(tc.tile_pool(name="vtp", bufs=1))
        ap_pool = ctx.enter_context(tc.tile_pool(name="ap", bufs=34))
        op_pool = ctx.enter_context(tc.tile_pool(name="op", bufs=1))
        gnp = ctx.enter_context(tc.tile_pool(name="gnp", bufs=1))
        oup = ctx.enter_context(tc.tile_pool(name="oup", bufs=1))
        gathp = ctx.enter_context(tc.tile_pool(name="gath", bufs=2))
        stgp = ctx.enter_context(tc.tile_pool(name="stg", bufs=2))
        recp = ctx.enter_context(tc.tile_pool(name="recp", bufs=2))
        rbcp = ctx.enter_context(tc.tile_pool(name="rbc", bufs=4))
        outp = ctx.enter_context(tc.tile_pool(name="outp", bufs=4))
        dramp = ctx.enter_context(tc.tile_pool(name="dramp", bufs=4, space="DRAM"))
        ps_sc = ctx.enter_context(tc.tile_pool(name="ps_sc", bufs=2, space="PSUM"))
        ps_av = ctx.enter_context(tc.tile_pool(name="ps_av", bufs=1, space="PSUM"))
        ps_fx = ctx.enter_context(tc.tile_pool(name="ps_fx", bufs=2, space="PSUM"))

        # ---- load x first (GN gates everything), then weights, then consts.
        # Batched [128, CCH, *] layouts: one DMA instruction each, 8KB+ rows.
        # x comes in two halves so bn_stats overlaps the second transfer.
        x_bt = xp.tile([128, CCH, N], bf16, tag="x")
        nc.sync.dma_start(out=x_bt[:, 0:2, :], in_=x_d[:, 0:2, :])
        nc.sync.dma_start(out=x_bt[:, 2:4, :], in_=x_d[:, 2:4, :])
        x_sb = [x_bt[:, cc, :] for cc in range(CCH)]
        mask_sb = consts.tile([128, 128], f32, tag="mask")
        nc.sync.dma_start(out=mask_sb, in_=mask_d[:, :])
        gnw_all = consts.tile([128, CCH], f32, tag="gnw")
        nc.sync.dma_start(out=gnw_all, in_=gnw_d[:, :])
        gnb_all = consts.tile([128, CCH], f32, tag="gnb")
        nc.sync.dma_start(out=gnb_all, in_=gnb_d[:, :])
        qkvw_bt = wqp.tile([128, CCH, 3 * C], bf16, tag="qw")
        nc.sync.dma_start(out=qkvw_bt, in_=qkvw_d[:, :, :])
        qkvw_sb = [qkvw_bt[:, cc, :] for cc in range(CCH)]
        qb_all = consts.tile([128, CCH], f32, tag="qb")
        nc.sync.dma_start(out=qb_all, in_=qb_d[:, :])
        kb_all = consts.tile([128, CCH], f32, tag="kb")
        nc.sync.dma_start(out=kb_all, in_=kb_d[:, :])
        eps_sb = consts.tile([128, 1], f32, tag="eps")
        nc.vector.memset(eps_sb, EPS)
        zero_sb = consts.tile([128, 1], f32, tag="zero")
        nc.vector.memset(zero_sb, 0.0)
        pb_sb = []
        for cc in range(CCH):
            t = consts.tile([128, 1], f32, tag=f"pb{cc}")
            nc.sync.dma_start(out=t, in_=pb_d[cc])
            pb_sb.append(t)
        pw_bt = consts.tile([128, CCH, C], bf16, tag="pw")
        nc.sync.dma_start(out=pw_bt, in_=pw_d[:, :, :])
        pw_sb = [pw_bt[:, cc, :] for cc in range(CCH)]
        id_sb = consts.tile([128, 128], bf16, tag="id128")
        nc.sync.dma_start(out=id_sb, in_=id_d[:, :])

        # ---- GroupNorm (stats batched across the 4 channel chunks) ----
        mv_all = gnp.tile([128, CCH, 2], f32, tag="mv")
        for cc in range(CCH):
            stats = gnp.tile([128, 2, 6], f32, tag=f"st{cc}")
            for sg in range(2):
                nc.vector.bn_stats(
                    out=stats[:, sg, :], in_=x_sb[cc][:, sg * 512 : (sg + 1) * 512]
                )
            nc.vector.bn_aggr(out=mv_all[:, cc, :], in_=stats)
        st2 = gnp.tile([128, CCH, 2], f32, tag="s2")
        nc.vector.tensor_copy(out=st2[:, :, 0], in_=mv_all[:, :, 0])
        mean_sq = gnp.tile([128, CCH], f32, tag="msq")
        nc.vector.tensor_mul(out=mean_sq, in0=mv_all[:, :, 0], in1=mv_all[:, :, 0])
        nc.vector.tensor_add(out=st2[:, :, 1], in0=mv_all[:, :, 1], in1=mean_sq)
        ps_st = ps_fx.tile([128, C], f32, tag="fx")
        dep_nop = nc.tensor.nop(hint="dep").ins
        dep_nop.ins = [nc.tensor.lower_ap(mask_sb), nc.tensor.lower_ap(st2[:, :, :])]
        nc.tensor.matmul(
            ps_st[:, 0 : CCH * 2],
            lhsT=mask_sb,
            rhs=st2.rearrange("p c two -> p (c two)"),
            start=True,
            stop=True,
        )
        gst = gnp.tile([128, CCH, 2], f32, tag="gstsb")
        nc.vector.tensor_copy(
            out=gst, in_=ps_st[:, 0 : CCH * 2].rearrange("p (c two) -> p c two", two=2)
        )
        gm2 = gnp.tile([128, CCH], f32, tag="g2")
        nc.vector.tensor_mul(out=gm2, in0=gst[:, :, 0], in1=gst[:, :, 0])
        gvar = gnp.tile([128, CCH], f32, tag="gv")
        nc.vector.tensor_sub(out=gvar, in0=gst[:, :, 1], in1=gm2)
        # rstd = exp(-0.5*ln(var+eps)): same ACT table set as attention exp.
        lnv = gnp.tile([128, CCH], f32, tag="lnv")
        nc.scalar.activation(out=lnv, in_=gvar, func=AF.Ln, bias=eps_sb)
        rstd = gnp.tile([128, CCH], f32, tag="rstd")
        nc.scalar.activation(out=rstd, in_=lnv, func=AF.Exp, scale=-0.5, bias=zero_sb)
        gscale = gnp.tile([128, CCH], f32, tag="gs")
        nc.vector.tensor_mul(out=gscale, in0=rstd, in1=gnw_all)
        t4 = gnp.tile([128, CCH], f32, tag="t4")
        nc.vector.tensor_mul(out=t4, in0=gst[:, :, 0], in1=gscale)
        gbias = gnp.tile([128, CCH], f32, tag="gb")
        nc.vector.tensor_sub(out=gbias, in0=gnb_all, in1=t4)
        xn_sb = []
        for cc in range(CCH):
            xn = xnp.tile([128, N], bf16, tag=f"xn{cc}")
            nc.vector.tensor_scalar(
                out=xn,
                in0=x_sb[cc],
                scalar1=gscale[:, cc : cc + 1],
                scalar2=gbias[:, cc : cc + 1],
                op0=OP.mult,
                op1=OP.add,
            )
            xn_sb.append(xn)

        # ---- stage emitters ----
        q_sb = [None] * CCH
        k_sb = [None] * CCH
        vt_sb = [None] * 8
        a_tiles = {}  # (p, mi, hh) -> sbuf bf16 [128, N]
        o_sb = []
        for p in range(CCH):
            o_tile = op_pool.tile([128, N], bf16, tag=f"o{p}")
            o_sb.append(o_tile)

        def emit_qk_group(p, which, nj):
            # one (q|k, nj) chunk of head pair p
            base, dest, brow = (
                (0, q_sb, qb_all) if which == "q" else (C, k_sb, kb_all)
            )
            if dest[p] is None:
                t = qkp.tile([128, N], bf16, tag=f"{which}{p}")
                dest[p] = t
            t = dest[p]
            ps = ps_fx.tile([128, C], f32, tag="fx")
            for cc in range(CCH):
                nc.tensor.matmul(
                    ps,
                    lhsT=(qkvw_sb[cc][:, base + p * 128 : base + (p + 1) * 128]),
                    rhs=(xn_sb[cc][:, nj * 512 : (nj + 1) * 512]),
                    start=(cc == 0),
                    stop=(cc == CCH - 1),
                )
            nc.vector.tensor_scalar(
                out=t[:, nj * 512 : (nj + 1) * 512],
                in0=ps,
                scalar1=brow[:, p : p + 1],
                scalar2=None,
                op0=OP.add,
            )

        def emit_qk(p):
            # k-nj1 last: scores(p, 0) needs q (both nj) and k-nj0 only
            for which, nj in (("q", 0), ("q", 1), ("k", 0), ("k", 1)):
                emit_qk_group(p, which, nj)

        def emit_vt(mi):
            # v^T for key chunk mi: [m partitions, head, 64 d + ones column]
            vt = vtp.tile([128, NH, HD + 1], bf16, tag=f"vt{mi}")
            nc.vector.memset(vt[:, :, HD : HD + 1], 1.0)
            ps = ps_fx.tile([128, C], f32, tag="fx")
            for cc in range(CCH):
                nc.tensor.matmul(
                    ps,
                    lhsT=(xn_sb[cc][:, mi * 128 : (mi + 1) * 128]),
                    rhs=(qkvw_sb[cc][:, 2 * C : 3 * C]),
                    start=(cc == 0),
                    stop=(cc == CCH - 1),
                )
            nc.vector.tensor_copy(
                out=vt[:, :, 0:HD], in_=ps.rearrange("p (h d) -> p h d", h=NH)
            )
            vt_sb[mi] = vt

        def emit_scores(p, mi):
            # S^T tiles for both heads of pair p, key chunk mi. The two heads
            # sit on PE row-tiles (0,0)/(64,0) and run concurrently.
            for nj in range(2):
                for hh in range(2):
                    if nj == 0:
                        ps_e = ps_sc.tile([128, N], f32, tag="sc")
                        at = ap_pool.tile([128, N], bf16, tag="a")
                        a_tiles[(p, mi, hh)] = (at, ps_e)
                    _, ps_e = a_tiles[(p, mi, hh)]
                    nc.tensor.matmul(
                        ps_e[:, nj * 512 : (nj + 1) * 512],
                        lhsT=(
                            k_sb[p][hh * 64 : (hh + 1) * 64, mi * 128 : (mi + 1) * 128]
                        ),
                        rhs=(
                            q_sb[p][hh * 64 : (hh + 1) * 64, nj * 512 : (nj + 1) * 512]
                        ),
                        start=True,
                        stop=True,
                    )
            for hh in range(2):
                at, ps_e = a_tiles[(p, mi, hh)]
                nc.scalar.activation(out=at, in_=ps_e, func=AF.Exp, bias=zero_sb)
                a_tiles[(p, mi, hh)] = (at, None)

        ou_sb = []
        for p in range(CCH):
            ou_tile = oup.tile([128, N], bf16, tag=f"ou{p}")
            ou_sb.append(ou_tile)
        # gath[0]: sum rows of pairs 0-1 (4 rows, base 0) for the batched
        # DVE reciprocal; pairs 2-3 use direct ACT chains at the tail.
        gath0 = gathp.tile([4, N], f32, tag="gath0")
        gath = [gath0]

        def emit_av_mms(p, nj, av_tiles, mis, start, stop):
            # av_tiles: [(tile, col_off) for hh in (0, 1)]; each [65, 512]
            # accumulation: rows 0-63 = A'V, row 64 = softmax denominator.
            for hh in range(2):
                h = 2 * p + hh
                av_t, co = av_tiles[hh]
                for j, mi in enumerate(mis):
                    nc.tensor.matmul(
                        av_t[0 : HD + 1, co : co + 512],
                        lhsT=vt_sb[mi][:, h, :],
                        rhs=a_tiles[(p, mi, hh)][0][:, nj * 512 : (nj + 1) * 512],
                        start=(start and j == 0),
                        stop=(stop and j == len(mis) - 1),
                    )

        def emit_release(p, nj, av_tiles):
            # Release AV PSUM: data rows -> ou_sb, sum row -> stage row ->
            # (DMA) gath row, for batched off-critical-path reciprocal.
            stg = stgp.tile([1, N], f32, tag="stg")
            for hh in range(2):
                av_t, co = av_tiles[hh]
                nc.vector.tensor_copy(
                    out=ou_sb[p][hh * 64 : (hh + 1) * 64, nj * 512 : (nj + 1) * 512],
                    in_=av_t[0:HD, co : co + 512],
                )
                nc.vector.tensor_copy(
                    out=stg[0:1, hh * 512 : (hh + 1) * 512],
                    in_=av_t[HD : HD + 1, co : co + 512],
                )
            nc.sync.dma_start(
                out=gath[p // 2][(p % 2) * 2 + nj : (p % 2) * 2 + nj + 1, :], in_=stg
            )

        def emit_av_release(p, nj, av_tiles):
            emit_av_mms(p, nj, av_tiles, range(8), True, True)
            emit_release(p, nj, av_tiles)

        def emit_norm_mult(p, rec_dram, base_row):
            # rec_dram rows base_row+nj hold [rec_e | rec_o] for pair p.
            # One DMA broadcasts both heads: src AP [hh=2, bcast 64, 512].
            for nj in range(2):
                rbc = rbcp.tile([128, 512], bf16, tag="rbc")
                for hh in range(2):
                    row = rec_dram[base_row + nj : base_row + nj + 1,
                                   hh * 512 : (hh + 1) * 512]
                    bcast = bass.AP(
                        tensor=row.tensor,
                        offset=row.offset,
                        ap=[[0, HD]] + [list(x) for x in row.ap[1:]],
                    )
                    nc.sync.dma_start(
                        out=rbc[hh * 64 : (hh + 1) * 64, :], in_=bcast
                    )
                nc.vector.tensor_mul(
                    out=o_sb[p][:, nj * 512 : (nj + 1) * 512],
                    in0=ou_sb[p][:, nj * 512 : (nj + 1) * 512],
                    in1=rbc,
                )

        def emit_release_direct(p, nj, av_tiles):
            # Tail variant: ou copies on DVE, and the reciprocal as a direct
            # ACT ln/exp chain straight off the PSUM sum rows (ACT is idle
            # once the exp stream ends; partition base 64 is 32-aligned).
            ls = recp.tile([1, N], f32, tag="lsum")
            for hh in range(2):
                av_t, co = av_tiles[hh]
                nc.scalar.activation(
                    out=ls[0:1, hh * 512 : (hh + 1) * 512],
                    in_=av_t[HD : HD + 1, co : co + 512],
                    func=AF.Ln,
                    bias=zero_sb[0:1],
                )
            rc = recp.tile([1, N], bf16, tag="rec")
            nc.scalar.activation(
                out=rc, in_=ls, func=AF.Exp, scale=-1.0, bias=zero_sb[0:1]
            )
            rd = dramp.tile([1, N], bf16, tag="recd")
            nc.sync.dma_start(out=rd, in_=rc)
            for hh in range(2):
                av_t, co = av_tiles[hh]
                nc.vector.tensor_copy(
                    out=ou_sb[p][hh * 64 : (hh + 1) * 64, nj * 512 : (nj + 1) * 512],
                    in_=av_t[0:HD, co : co + 512],
                )
            emit_norm_mult_nj(p, nj, rd)

        def emit_norm_mult_nj(p, nj, rec_dram):
            rbc = rbcp.tile([128, 512], bf16, tag="rbc")
            for hh in range(2):
                row = rec_dram[0:1, hh * 512 : (hh + 1) * 512]
                bcast = bass.AP(
                    tensor=row.tensor,
                    offset=row.offset,
                    ap=[[0, HD]] + [list(x) for x in row.ap[1:]],
                )
                nc.sync.dma_start(out=rbc[hh * 64 : (hh + 1) * 64, :], in_=bcast)
            nc.vector.tensor_mul(
                out=o_sb[p][:, nj * 512 : (nj + 1) * 512],
                in0=ou_sb[p][:, nj * 512 : (nj + 1) * 512],
                in1=rbc,
            )

        def av_pair_tiles():
            av_t = ps_av.tile([128, N], f32, tag="av")
            return [(av_t, 0), (av_t, 512)]

        # ---- pipeline ----
        # Steady state per pair p: the previous pair's AV(nj=0) matmuls are
        # spread one key-chunk at a time AHEAD of each scores step (so the
        # in-order PE queue always has ready work and HAM stays warm);
        # AV(nj=1) re-reads the same a-tiles in a batch at the pair boundary.
        # qk for pair p+1 is spread across the back half of the pair.
        emit_qk(0)
        for mi in range(8):
            emit_scores(0, mi)
            emit_vt(mi)
        emit_qk(1)
        av_cur = None
        for p in (1, 2):
            for mi in range(8):
                if mi == 0:
                    av_cur = av_pair_tiles()
                emit_av_mms(p - 1, 0, av_cur, [mi], mi == 0, mi == 7)
                emit_scores(p, mi)
                if mi >= 4:
                    emit_qk_group(p + 1, "qk"[mi % 2], (mi - 4) // 2)
            emit_release(p - 1, 0, av_cur)
            av_nj1 = av_pair_tiles()
            emit_av_mms(p - 1, 1, av_nj1, range(8), True, True)
            emit_release(p - 1, 1, av_nj1)
        # pair 3: pair-2's AV is compressed into the front half so its
        # (mid-stream) ACT reciprocal chains free PSUM for pair-3's own AV,
        # which then starts inside the exp window instead of after it.
        for mi in range(8):
            if mi == 0:
                av_cur = av_pair_tiles()
            if mi < 4:
                emit_av_mms(2, 0, av_cur, [2 * mi, 2 * mi + 1], mi == 0, mi == 3)
            elif mi == 4:
                fx_e1 = ps_fx.tile([128, 512], f32, tag="fx")
                fx_o1 = ps_fx.tile([128, 512], f32, tag="fx")
                av21 = [(fx_e1, 0), (fx_o1, 0)]
                emit_av_mms(2, 1, av21, range(0, 4), True, False)
                emit_release_direct(2, 0, av_cur)
            elif mi == 5:
                emit_av_mms(2, 1, av21, range(4, 8), False, True)
            elif mi == 6:
                emit_release_direct(2, 1, av21)
                av3 = av_pair_tiles()
                emit_av_mms(3, 0, av3, range(0, 2), True, False)
            elif mi == 7:
                emit_av_mms(3, 0, av3, range(2, 6), False, False)
            emit_scores(3, mi)
            if mi == 3:
                # pairs 0-1 reciprocal, batched [4, N] (one slow DVE op),
                # after the pair-2 release copies are already queued.
                rec1 = recp.tile([4, N], bf16, tag="rec")
                nc.vector.reciprocal(out=rec1, in_=gath[0])
                rec1_d = dramp.tile([4, N], bf16, tag="recd")
                nc.sync.dma_start(out=rec1_d, in_=rec1)
                emit_norm_mult(0, rec1_d, 0)
                emit_norm_mult(1, rec1_d, 2)
        # ---- tail ----
        emit_av_mms(3, 0, av3, range(6, 8), False, True)
        emit_release_direct(3, 0, av3)
        fx_e = ps_fx.tile([128, 512], f32, tag="fx")
        fx_o = ps_fx.tile([128, 512], f32, tag="fx")
        av3_1 = [(fx_e, 0), (fx_o, 0)]
        emit_av_mms(3, 1, av3_1, range(8), True, True)
        emit_release_direct(3, 1, av3_1)
        # proj accumulators: all 8 PSUM banks are free (or freeing) now.
        pj = {}
        for oc in range(CCH):
            if oc < 2:
                t = ps_sc.tile([128, N], f32, tag="sc")
            elif oc == 2:
                a2 = ps_fx.tile([128, 512], f32, tag="fx")
                b2 = ps_fx.tile([128, 512], f32, tag="fx")
                pj[(2, 0)], pj[(2, 1)] = a2, b2
                continue
            else:
                t = ps_av.tile([128, N], f32, tag="av")
            pj[(oc, 0)], pj[(oc, 1)] = t[:, 0:512], t[:, 512:1024]

        def emit_proj_cc(ccs, start, stop):
            for oc in range(CCH):
                for nj in range(2):
                    for j, cc in enumerate(ccs):
                        nc.tensor.matmul(
                            pj[(oc, nj)],
                            lhsT=(pw_sb[cc][:, oc * 128 : (oc + 1) * 128]),
                            rhs=(o_sb[cc][:, nj * 512 : (nj + 1) * 512]),
                            start=(start and j == 0),
                            stop=(stop and j == len(ccs) - 1),
                        )

        # Residual rides the accumulation as an identity matmul (x is bf16),
        # so the copy-out is a plain ACT bias-add on the (idle) ACT engine.
        for oc in range(CCH):
            for nj in range(2):
                nc.tensor.matmul(
                    pj[(oc, nj)],
                    lhsT=id_sb,
                    rhs=x_sb[oc][:, nj * 512 : (nj + 1) * 512],
                    start=True,
                    stop=False,
                )
        # partials over the already-normalized pairs 0-1, overlapping the
        # pairs-2/3 reciprocal chain on ACT/DVE
        emit_proj_cc((0, 1), False, False)
        emit_proj_cc((2,), False, False)
        emit_proj_cc((3,), False, True)
        for oc in range(CCH):
            ot = outp.tile([128, N], f32, tag="ot")
            for nj in range(2):
                nc.scalar.activation(
                    out=ot[:, nj * 512 : (nj + 1) * 512],
                    in_=pj[(oc, nj)],
                    func=AF.Identity,
                    bias=pb_sb[oc],
                )
            nc.sync.dma_start(out=out_d[oc], in_=ot)

    _add_dram_raw_waits(nc)
    _split_lw_waits(nc)
    return nc


def _add_dram_raw_waits(nc):
    """Tile does not order DMA-after-DMA RAW hazards through DRAM: a DMA
    reading a DRAM scratch tensor gets no wait on the (different-queue) DMA
    that wrote it.  Insert an explicit sem-ge wait on the writer's completion
    semaphore value before each such reader."""
    from concourse import mybir

    for blk in nc.m.functions[0].blocks:
        cum = {}  # semaphore id -> accumulated update value (SP issue order)
        writer = {}  # memsetref name -> (sem id, ant_name, value after write)
        out = []
        for inst in blk.instructions:
            is_dma = type(inst).__name__ == "InstDMACopy"
            si = getattr(inst, "sync_info", None)
            if is_dma:
                for a in inst.ins:
                    n = getattr(a, "memsetref", None)
                    if n in writer:
                        sem_id, ant, val = writer.pop(n)
                        w = mybir.SyncWait(
                            sync_type="semaphore",
                            id=sem_id,
                            ant_name=ant,
                            wait_mode="sem-ge-imm",
                            wait_value=val,
                        )
                        out.append(
                            mybir.InstEventSemaphore(
                                name=f"{inst.name}_dramraw",
                                engine=inst.engine,
                                ins=[],
                                outs=[],
                                sync_info=mybir.SyncInfo(on_wait=[w], on_update=[]),
                            )
                        )
            if si is not None:
                for u in si.on_update or []:
                    if getattr(u, "update_mode", "") == "sem-add-imm":
                        cum[u.id] = cum.get(u.id, 0) + u.update_value
            if is_dma and si is not None:
                for u in si.on_update or []:
                    if getattr(u, "update_mode", "") == "sem-add-imm":
                        for a in inst.outs:
                            n = getattr(a, "memsetref", None)
                            if n:
                                writer[n] = (u.id, u.ant_name, cum[u.id])
            out.append(inst)
        blk.instructions = out


def _split_lw_waits(nc):
    """This walrus build accepts only ONE sync-wait command per engine
    instruction; Tile can attach several. First coalesce sem-ge waits on the
    same semaphore (keep the max threshold), then hoist each excess wait onto
    its own pure sem-wait instruction inserted just before, in queue order."""
    from concourse import mybir

    def coalesce(waits):
        best, rest = {}, []
        for w in waits:
            if (
                getattr(w, "sync_type", "") == "semaphore"
                and getattr(w, "wait_mode", "") == "sem-ge-imm"
            ):
                k = w.id
                if k not in best or best[k].wait_value < w.wait_value:
                    best[k] = w
            else:
                rest.append(w)
        return list(best.values()) + rest

    for blk in nc.m.functions[0].blocks:
        out = []
        for inst in blk.instructions:
            si = getattr(inst, "sync_info", None)
            if si is not None and si.on_wait and len(si.on_wait) > 1:
                inst.sync_info = si = mybir.SyncInfo(
                    on_wait=coalesce(si.on_wait), on_update=list(si.on_update or [])
                )
            if (
                si is not None
                and si.on_wait
                and len(si.on_wait) > 1
                and type(inst).__name__ != "InstEventSemaphore"
            ):
                waits = list(si.on_wait)
                for j, w in enumerate(waits[:-1]):
                    sem = mybir.InstEventSemaphore(
                        name=f"{inst.name}_wsplit{j}",
                        engine=inst.engine,
                        ins=[],
                        outs=[],
                        sync_info=mybir.SyncInfo(on_wait=[w], on_update=[]),
                    )
                    out.append(sem)
                inst.sync_info = mybir.SyncInfo(
                    on_wait=waits[-1:], on_update=list(si.on_update or [])
                )
            out.append(inst)
        blk.instructions = out


def kernel(x, gn_w, gn_b, qkv_w, qkv_b, proj_w, proj_b):
    from concourse.bass_utils import run_bass_kernel_spmd

    B = x.shape[0]
    assert B == NCORES
    if "nc" not in _CACHE:
        _CACHE["nc"] = _build_program()
    nc = _CACHE["nc"]

    scale = float(HD) ** -0.5
    xf = np.ascontiguousarray(
        x.reshape(B, CCH, 128, N).transpose(0, 2, 1, 3).astype(ml_dtypes.bfloat16)
    )
    # layout [128, CCH, 3C]: for input chunk cc the 3C columns are [q | k | v]
    qkv_wT = np.ascontiguousarray(
        np.concatenate(
            [
                qkv_w[0:C].T.reshape(CCH, 128, C),
                (qkv_w[C : 2 * C] * scale).T.reshape(CCH, 128, C),
                qkv_w[2 * C : 3 * C].T.reshape(CCH, 128, C),
            ],
            axis=2,
        )
        .transpose(1, 0, 2)
        .astype(ml_dtypes.bfloat16)
    )
    qb = np.ascontiguousarray(qkv_b[0:C].reshape(CCH, 128).T.astype(np.float32))
    kb = np.ascontiguousarray(
        (qkv_b[C : 2 * C] * scale).reshape(CCH, 128).T.astype(np.float32)
    )
    vb = qkv_b[2 * C : 3 * C]
    pw_T = np.ascontiguousarray(
        proj_w.T.reshape(CCH, 128, C).transpose(1, 0, 2).astype(ml_dtypes.bfloat16)
    )
    pb = np.ascontiguousarray(
        (proj_b + proj_w.astype(np.float64) @ vb.astype(np.float64))
        .astype(np.float32)
        .reshape(CCH, 128, 1)
    )
    gnw = np.ascontiguousarray(gn_w.reshape(CCH, 128).T.astype(np.float32))
    gnb = np.ascontiguousarray(gn_b.reshape(CCH, 128).T.astype(np.float32))
    id128 = np.ascontiguousarray(np.eye(128, dtype=np.float32).astype(ml_dtypes.bfloat16))
    mask = np.zeros((128, 128), dtype=np.float32)
    for g in range(2):
        mask[g * 64 : (g + 1) * 64, g * 64 : (g + 1) * 64] = 1.0 / 64.0

    in_maps = []
    for i in range(NCORES):
        in_maps.append(
            {
                "x": xf[i],
                "qkv_wT": qkv_wT,
                "qb": qb,
                "kb": kb,
                "proj_wT": pw_T,
                "pb": pb,
                "gnw": gnw,
                "gnb": gnb,
                "gn_mask": mask,
                "id128": id128,
            }
        )

    tmpdir = os.environ.get("BASS_TMPDIR")
    if tmpdir:
        os.makedirs(tmpdir, exist_ok=True)
    res = run_bass_kernel_spmd(
        nc,
        in_maps,
        list(range(NCORES)),
        trace=bool(os.environ.get("BASS_TRACE")),
        tmpdir=tmpdir,
    )
    LAST["exec_time_ns"] = res.exec_time_ns
    LAST["results"] = res
    out = np.stack([res.results[i]["out"] for i in range(NCORES)], axis=0)
    return out.reshape(B, C, 32, 32).astype(x.dtype)


# revision 51
# speedup vs baseline: 1.1590x; 1.1590x over previous
"""AttentionBlock kernel for 8 Trainium2 NeuronCores.

Reference op: GroupNorm(8 groups) -> 1x1 conv qkv -> 8-head attention over
1024 spatial positions -> 1x1 conv proj -> residual.   Shapes (full):
x [8, 512, 32, 32]; qkv_w [1536, 512]; proj_w [512, 512].

Sharding: pure data-parallel over batch - one batch element per core.

Per-core design (v2, exp-stream pipelined):
  - x loaded in bf16 (halves input DMA; residual quantization ~4e-3 abs).
  - q/k biases folded into the PSUM->SBUF copies (DVE tensor_scalar add);
    the 1/sqrt(hd) attention scale is folded into the k weights+bias on
    host.  v bias is folded into the proj bias on host (softmax weights
    sum to 1, so  proj_w @ vb  is an exact per-channel constant).
  - Scores computed transposed (S^T = K^T Q) per (pair, key-chunk, head):
    two matmuls (head even / head odd) occupy row-tiles (0,0)/(64,0) of
    the PE array and run concurrently.
  - exp on ACT engine at FD=1024 per tile (the engine floor: 64 tiles x
    ~1.15us ~= 74us governs the kernel); PSUM scores double-buffered so
    the ACT stream never stalls.
  - AV rides the ones-column trick (65th row = softmax denominator),
    accumulated per (hh, nj) in batched matmul groups.
  - Softmax normalization: AV PSUM is released by cheap DVE copies (data
    rows -> ou_sb, sum row -> a staging row that is DMA-gathered into a
    [4,N] tile).  The reciprocal then runs OFF the critical path: one DVE
    reciprocal [4,N] covering pairs 0-1 mid-stream, and an ACT ln/exp
    chain for pairs 2-3 at the tail (ACT is idle once the exp stream
    ends).  Broadcast back via DRAM bounce, final o = ou * rec on DVE.
"""

import os

import numpy as np
import ml_dtypes

NCORES = 8
C = 512
N = 1024  # 32*32 spatial
NH = 8
HD = 64  # head dim
CCH = 4  # channel chunks of 128
EPS = 1e-5

_CACHE = {}
LAST = {"exec_time_ns": None, "results": None}


def _build_program():
    import concourse.bass as bass
    import concourse.tile as tile
    from concourse import mybir

    f32 = mybir.dt.float32
    bf16 = mybir.dt.bfloat16
    AF = mybir.ActivationFunctionType
    OP = mybir.AluOpType

    nc = bass.Bass()

    # ---- DRAM parameters (per core). Host pre-reshapes/pre-transposes. ----
    x_d = nc.declare_dram_parameter("x", [128, CCH, N], bf16, isOutput=False)
    qkvw_d = nc.declare_dram_parameter("qkv_wT", [128, CCH, 3 * C], bf16, isOutput=False)
    qb_d = nc.declare_dram_parameter("qb", [128, CCH], f32, isOutput=False)
    kb_d = nc.declare_dram_parameter("kb", [128, CCH], f32, isOutput=False)
    pw_d = nc.declare_dram_parameter("proj_wT", [128, CCH, C], bf16, isOutput=False)
    pb_d = nc.declare_dram_parameter("pb", [CCH, 128, 1], f32, isOutput=False)
    gnw_d = nc.declare_dram_parameter("gnw", [128, CCH], f32, isOutput=False)
    gnb_d = nc.declare_dram_parameter("gnb", [128, CCH], f32, isOutput=False)
    mask_d = nc.declare_dram_parameter("gn_mask", [128, 128], f32, isOutput=False)
    id_d = nc.declare_dram_parameter("id128", [128, 128], bf16, isOutput=False)
    out_d = nc.declare_dram_parameter("out", [CCH, 128, N], f32, isOutput=True)

    from contextlib import ExitStack

    with (
        nc.allow_low_precision(reason="bf16 tiles feed full-speed matmuls"),
        tile.TileContext(nc) as tc,
        ExitStack() as ctx,
    ):
        consts = ctx.enter_context(tc.tile_pool(name="consts", bufs=1))
        xp = ctx.enter_context(tc.tile_pool(name="xp", bufs=1))
        wqp = ctx.enter_context(tc.tile_pool(name="wqp", bufs=1))
        xnp = ctx.enter_context(tc.tile_pool(name="xnp", bufs=1))
        qkp = ctx.enter_context(tc.tile_pool(name="qkp", bufs=1))
        vtp = ctx.enter_context(tc.tile_pool(name="vtp", bufs=1))
        ap_pool = ctx.enter_context(tc.tile_pool(name="ap", bufs=34))
        op_pool = ctx.enter_context(tc.tile_pool(name="op", bufs=1))
        gnp = ctx.enter_context(tc.tile_pool(name="gnp", bufs=1))
        oup = ctx.enter_context(tc.tile_pool(name="oup", bufs=1))
        gathp = ctx.enter_context(tc.tile_pool(name="gath", bufs=2))
        stgp = ctx.enter_context(tc.tile_pool(name="stg", bufs=2))
        recp = ctx.enter_context(tc.tile_pool(name="recp", bufs=2))
        rbcp = ctx.enter_context(tc.tile_pool(name="rbc", bufs=4))
        outp = ctx.enter_context(tc.tile_pool(name="outp", bufs=4))
        dramp = ctx.enter_context(tc.tile_pool(name="dramp", bufs=4, space="DRAM"))
        ps_sc = ctx.enter_context(tc.tile_pool(name="ps_sc", bufs=2, space="PSUM"))
        ps_av = ctx.enter_context(tc.tile_pool(name="ps_av", bufs=1, space="PSUM"))
        ps_fx = ctx.enter_context(tc.tile_pool(name="ps_fx", bufs=2, space="PSUM"))

        # ---- load x first (GN gates everything), then weights, then consts.
        # Batched [128, CCH, *] layouts: one DMA instruction each, 8KB+ rows.
        # x comes in two halves so bn_stats overlaps the second transfer.
        x_bt = xp.tile([128, CCH, N], bf16, tag="x")
        nc.sync.dma_start(out=x_bt[:, 0:2, :], in_=x_d[:, 0:2, :])
        nc.sync.dma_start(out=x_bt[:, 2:4, :], in_=x_d[:, 2:4, :])
        x_sb = [x_bt[:, cc, :] for cc in range(CCH)]
        mask_sb = consts.tile([128, 128], f32, tag="mask")
        nc.sync.dma_start(out=mask_sb, in_=mask_d[:, :])
        gnw_all = consts.tile([128, CCH], f32, tag="gnw")
        nc.sync.dma_start(out=gnw_all, in_=gnw_d[:, :])
        gnb_all = consts.tile([128, CCH], f32, tag="gnb")
        nc.sync.dma_start(out=gnb_all, in_=gnb_d[:, :])
        qkvw_bt = wqp.tile([128, CCH, 3 * C], bf16, tag="qw")
        nc.sync.dma_start(out=qkvw_bt, in_=qkvw_d[:, :, :])
        qkvw_sb = [qkvw_bt[:, cc, :] for cc in range(CCH)]
        qb_all = consts.tile([128, CCH], f32, tag="qb")
        nc.sync.dma_start(out=qb_all, in_=qb_d[:, :])
        kb_all = consts.tile([128, CCH], f32, tag="kb")
        nc.sync.dma_start(out=kb_all, in_=kb_d[:, :])
        eps_sb = consts.tile([128, 1], f32, tag="eps")
        nc.vector.memset(eps_sb, EPS)
        zero_sb = consts.tile([128, 1], f32, tag="zero")
        nc.vector.memset(zero_sb, 0.0)
        pb_sb = []
        for cc in range(CCH):
            t = consts.tile([128, 1], f32, tag=f"pb{cc}")
            nc.sync.dma_start(out=t, in_=pb_d[cc])
            pb_sb.append(t)
        pw_bt = consts.tile([128, CCH, C], bf16, tag="pw")
        nc.sync.dma_start(out=pw_bt, in_=pw_d[:, :, :])
        pw_sb = [pw_bt[:, cc, :] for cc in range(CCH)]
        id_sb = consts.tile([128, 128], bf16, tag="id128")
        nc.sync.dma_start(out=id_sb, in_=id_d[:, :])

        # ---- GroupNorm (stats batched across the 4 channel chunks) ----
        mv_all = gnp.tile([128, CCH, 2], f32, tag="mv")
        for cc in range(CCH):
            stats = gnp.tile([128, 2, 6], f32, tag=f"st{cc}")
            for sg in range(2):
                nc.vector.bn_stats(
                    out=stats[:, sg, :], in_=x_sb[cc][:, sg * 512 : (sg + 1) * 512]
                )
            nc.vector.bn_aggr(out=mv_all[:, cc, :], in_=stats)
        st2 = gnp.tile([128, CCH, 2], f32, tag="s2")
        nc.vector.tensor_copy(out=st2[:, :, 0], in_=mv_all[:, :, 0])
        mean_sq = gnp.tile([128, CCH], f32, tag="msq")
        nc.vector.tensor_mul(out=mean_sq, in0=mv_all[:, :, 0], in1=mv_all[:, :, 0])
        nc.vector.tensor_add(out=st2[:, :, 1], in0=mv_all[:, :, 1], in1=mean_sq)
        ps_st = ps_fx.tile([128, C], f32, tag="fx")
        dep_nop = nc.tensor.nop(hint="dep").ins
        dep_nop.ins = [nc.tensor.lower_ap(mask_sb), nc.tensor.lower_ap(st2[:, :, :])]
        nc.tensor.matmul(
            ps_st[:, 0 : CCH * 2],
            lhsT=mask_sb,
            rhs=st2.rearrange("p c two -> p (c two)"),
            start=True,
            stop=True,
        )
        # PE warm-up: dummy matmuls during the GN math so the HAM clock-gate
        # reaches 8/8 before the qkv/scores lead-in (results unused).
        warm_ps = ps_fx.tile([128, C], f32, tag="fx")
        for w in range(6):
            nc.tensor.matmul(
                warm_ps,
                lhsT=id_sb,
                rhs=x_sb[0][:, 0:512],
                start=(w == 0),
                stop=(w == 5),
            )
        gst = gnp.tile([128, CCH, 2], f32, tag="gstsb")
        nc.vector.tensor_copy(
            out=gst, in_=ps_st[:, 0 : CCH * 2].rearrange("p (c two) -> p c two", two=2)
        )
        gm2 = gnp.tile([128, CCH], f32, tag="g2")
        nc.vector.tensor_mul(out=gm2, in0=gst[:, :, 0], in1=gst[:, :, 0])
        gvar = gnp.tile([128, CCH], f32, tag="gv")
        nc.vector.tensor_sub(out=gvar, in0=gst[:, :, 1], in1=gm2)
        # rstd = exp(-0.5*ln(var+eps)): same ACT table set as attention exp.
        lnv = gnp.tile([128, CCH], f32, tag="lnv")
        nc.scalar.activation(out=lnv, in_=gvar, func=AF.Ln, bias=eps_sb)
        rstd = gnp.tile([128, CCH], f32, tag="rstd")
        nc.scalar.activation(out=rstd, in_=lnv, func=AF.Exp, scale=-0.5, bias=zero_sb)
        gscale = gnp.tile([128, CCH], f32, tag="gs")
        nc.vector.tensor_mul(out=gscale, in0=rstd, in1=gnw_all)
        t4 = gnp.tile([128, CCH], f32, tag="t4")
        nc.vector.tensor_mul(out=t4, in0=gst[:, :, 0], in1=gscale)
        gbias = gnp.tile([128, CCH], f32, tag="gb")
        nc.vector.tensor_sub(out=gbias, in0=gnb_all, in1=t4)
        xn_sb = []
        for cc in range(CCH):
            xn = xnp.tile([128, N], bf16, tag=f"xn{cc}")
            nc.vector.tensor_scalar(
                out=xn,
                in0=x_sb[cc],
                scalar1=gscale[:, cc : cc + 1],
                scalar2=gbias[:, cc : cc + 1],
                op0=OP.mult,
                op1=OP.add,
            )
            xn_sb.append(xn)

        # ---- stage emitters ----
        q_sb = [None] * CCH
        k_sb = [None] * CCH
        vt_sb = [None] * 8
        a_tiles = {}  # (p, mi, hh) -> sbuf bf16 [128, N]
        o_sb = []
        for p in range(CCH):
            o_tile = op_pool.tile([128, N], bf16, tag=f"o{p}")
            o_sb.append(o_tile)

        def emit_qk_group(p, which, nj):
            # one (q|k, nj) chunk of head pair p
            base, dest, brow = (
                (0, q_sb, qb_all) if which == "q" else (C, k_sb, kb_all)
            )
            if dest[p] is None:
                t = qkp.tile([128, N], bf16, tag=f"{which}{p}")
                dest[p] = t
            t = dest[p]
            ps = ps_fx.tile([128, C], f32, tag="fx")
            for cc in range(CCH):
                nc.tensor.matmul(
                    ps,
                    lhsT=(qkvw_sb[cc][:, base + p * 128 : base + (p + 1) * 128]),
                    rhs=(xn_sb[cc][:, nj * 512 : (nj + 1) * 512]),
                    start=(cc == 0),
                    stop=(cc == CCH - 1),
                )
            nc.vector.tensor_scalar(
                out=t[:, nj * 512 : (nj + 1) * 512],
                in0=ps,
                scalar1=brow[:, p : p + 1],
                scalar2=None,
                op0=OP.add,
            )

        def emit_qk(p):
            # k-nj1 last: scores(p, 0) needs q (both nj) and k-nj0 only
            for which, nj in (("q", 0), ("q", 1), ("k", 0), ("k", 1)):
                emit_qk_group(p, which, nj)

        def emit_vt(mi):
            # v^T for key chunk mi: [m partitions, head, 64 d + ones column]
            vt = vtp.tile([128, NH, HD + 1], bf16, tag=f"vt{mi}")
            nc.vector.memset(vt[:, :, HD : HD + 1], 1.0)
            ps = ps_fx.tile([128, C], f32, tag="fx")
            for cc in range(CCH):
                nc.tensor.matmul(
                    ps,
                    lhsT=(xn_sb[cc][:, mi * 128 : (mi + 1) * 128]),
                    rhs=(qkvw_sb[cc][:, 2 * C : 3 * C]),
                    start=(cc == 0),
                    stop=(cc == CCH - 1),
                )
            nc.vector.tensor_copy(
                out=vt[:, :, 0:HD], in_=ps.rearrange("p (h d) -> p h d", h=NH)
            )
            vt_sb[mi] = vt

        def emit_scores(p, mi):
            # S^T tiles for both heads of pair p, key chunk mi. The two heads
            # sit on PE row-tiles (0,0)/(64,0) and run concurrently.
            for nj in range(2):
                for hh in range(2):
                    if nj == 0:
                        ps_e = ps_sc.tile([128, N], f32, tag="sc")
                        at = ap_pool.tile([128, N], bf16, tag="a")
                        a_tiles[(p, mi, hh)] = (at, ps_e)
                    _, ps_e = a_tiles[(p, mi, hh)]
                    nc.tensor.matmul(
                        ps_e[:, nj * 512 : (nj + 1) * 512],
                        lhsT=(
                            k_sb[p][hh * 64 : (hh + 1) * 64, mi * 128 : (mi + 1) * 128]
                        ),
                        rhs=(
                            q_sb[p][hh * 64 : (hh + 1) * 64, nj * 512 : (nj + 1) * 512]
                        ),
                        start=True,
                        stop=True,
                    )
            for hh in range(2):
                at, ps_e = a_tiles[(p, mi, hh)]
                nc.scalar.activation(out=at, in_=ps_e, func=AF.Exp, bias=zero_sb)
                a_tiles[(p, mi, hh)] = (at, None)

        ou_sb = []
        for p in range(CCH):
            ou_tile = oup.tile([128, N], bf16, tag=f"ou{p}")
            ou_sb.append(ou_tile)
        # gath[0]: sum rows of pairs 0-1 (4 rows, base 0) for the batched
        # DVE reciprocal; pairs 2-3 use direct ACT chains at the tail.
        gath0 = gathp.tile([4, N], f32, tag="gath0")
        gath = [gath0]

        def emit_av_mms(p, nj, av_tiles, mis, start, stop):
            # av_tiles: [(tile, col_off) for hh in (0, 1)]; each [65, 512]
            # accumulation: rows 0-63 = A'V, row 64 = softmax denominator.
            for hh in range(2):
                h = 2 * p + hh
                av_t, co = av_tiles[hh]
                for j, mi in enumerate(mis):
                    nc.tensor.matmul(
                        av_t[0 : HD + 1, co : co + 512],
                        lhsT=vt_sb[mi][:, h, :],
                        rhs=a_tiles[(p, mi, hh)][0][:, nj * 512 : (nj + 1) * 512],
                        start=(start and j == 0),
                        stop=(stop and j == len(mis) - 1),
                    )

        def emit_release(p, nj, av_tiles):
            # Release AV PSUM: data rows -> ou_sb, sum row -> stage row ->
            # (DMA) gath row, for batched off-critical-path reciprocal.
            stg = stgp.tile([1, N], f32, tag="stg")
            for hh in range(2):
                av_t, co = av_tiles[hh]
                nc.vector.tensor_copy(
                    out=ou_sb[p][hh * 64 : (hh + 1) * 64, nj * 512 : (nj + 1) * 512],
                    in_=av_t[0:HD, co : co + 512],
                )
                nc.vector.tensor_copy(
                    out=stg[0:1, hh * 512 : (hh + 1) * 512],
                    in_=av_t[HD : HD + 1, co : co + 512],
                )
            nc.sync.dma_start(
                out=gath[p // 2][(p % 2) * 2 + nj : (p % 2) * 2 + nj + 1, :], in_=stg
            )

        def emit_av_release(p, nj, av_tiles):
            emit_av_mms(p, nj, av_tiles, range(8), True, True)
            emit_release(p, nj, av_tiles)

        def emit_norm_mult(p, rec_dram, base_row):
            # rec_dram rows base_row+nj hold [rec_e | rec_o] for pair p.
            # One DMA broadcasts both heads: src AP [hh=2, bcast 64, 512].
            for nj in range(2):
                rbc = rbcp.tile([128, 512], bf16, tag="rbc")
                for hh in range(2):
                    row = rec_dram[base_row + nj : base_row + nj + 1,
                                   hh * 512 : (hh + 1) * 512]
                    bcast = bass.AP(
                        tensor=row.tensor,
                        offset=row.offset,
                        ap=[[0, HD]] + [list(x) for x in row.ap[1:]],
                    )
                    nc.sync.dma_start(
                        out=rbc[hh * 64 : (hh + 1) * 64, :], in_=bcast
                    )
                nc.vector.tensor_mul(
                    out=o_sb[p][:, nj * 512 : (nj + 1) * 512],
                    in0=ou_sb[p][:, nj * 512 : (nj + 1) * 512],
                    in1=rbc,
                )

        def emit_release_direct(p, nj, av_tiles):
            # Tail variant: ou copies on DVE, and the reciprocal as a direct
            # ACT ln/exp chain straight off the PSUM sum rows (ACT is idle
            # once the exp stream ends; partition base 64 is 32-aligned).
            ls = recp.tile([1, N], f32, tag="lsum")
            for hh in range(2):
                av_t, co = av_tiles[hh]
                nc.scalar.activation(
                    out=ls[0:1, hh * 512 : (hh + 1) * 512],
                    in_=av_t[HD : HD + 1, co : co + 512],
                    func=AF.Ln,
                    bias=zero_sb[0:1],
                )
            rc = recp.tile([1, N], bf16, tag="rec")
            nc.scalar.activation(
                out=rc, in_=ls, func=AF.Exp, scale=-1.0, bias=zero_sb[0:1]
            )
            rd = dramp.tile([1, N], bf16, tag="recd")
            nc.sync.dma_start(out=rd, in_=rc)
            for hh in range(2):
                av_t, co = av_tiles[hh]
                nc.vector.tensor_copy(
                    out=ou_sb[p][hh * 64 : (hh + 1) * 64, nj * 512 : (nj + 1) * 512],
                    in_=av_t[0:HD, co : co + 512],
                )
            emit_norm_mult_nj(p, nj, rd)

        def emit_norm_mult_nj(p, nj, rec_dram):
            rbc = rbcp.tile([128, 512], bf16, tag="rbc")
            for hh in range(2):
                row = rec_dram[0:1, hh * 512 : (hh + 1) * 512]
                bcast = bass.AP(
                    tensor=row.tensor,
                    offset=row.offset,
                    ap=[[0, HD]] + [list(x) for x in row.ap[1:]],
                )
                nc.sync.dma_start(out=rbc[hh * 64 : (hh + 1) * 64, :], in_=bcast)
            nc.vector.tensor_mul(
                out=o_sb[p][:, nj * 512 : (nj + 1) * 512],
                in0=ou_sb[p][:, nj * 512 : (nj + 1) * 512],
                in1=rbc,
            )

        def av_pair_tiles():
            av_t = ps_av.tile([128, N], f32, tag="av")
            return [(av_t, 0), (av_t, 512)]

        # ---- pipeline ----
        # Steady state per pair p: the previous pair's AV(nj=0) matmuls are
        # spread one key-chunk at a time AHEAD of each scores step (so the
        # in-order PE queue always has ready work and HAM stays warm);
        # AV(nj=1) re-reads the same a-tiles in a batch at the pair boundary.
        # qk for pair p+1 is spread across the back half of the pair.
        emit_qk(0)
        for mi in range(8):
            emit_scores(0, mi)
            emit_vt(mi)
        emit_qk(1)
        av_cur = None
        for p in (1, 2):
            for mi in range(8):
                if mi == 0:
                    av_cur = av_pair_tiles()
                emit_av_mms(p - 1, 0, av_cur, [mi], mi == 0, mi == 7)
                emit_scores(p, mi)
                if mi >= 4:
                    emit_qk_group(p + 1, "qk"[mi % 2], (mi - 4) // 2)
            emit_release(p - 1, 0, av_cur)
            av_nj1 = av_pair_tiles()
            emit_av_mms(p - 1, 1, av_nj1, range(8), True, True)
            emit_release(p - 1, 1, av_nj1)
        # pair 3: pair-2's AV is compressed into the front half so its
        # (mid-stream) ACT reciprocal chains free PSUM for pair-3's own AV,
        # which then starts inside the exp window instead of after it.
        for mi in range(8):
            if mi == 0:
                av_cur = av_pair_tiles()
            if mi < 4:
                emit_av_mms(2, 0, av_cur, [2 * mi, 2 * mi + 1], mi == 0, mi == 3)
            elif mi == 4:
                fx_e1 = ps_fx.tile([128, 512], f32, tag="fx")
                fx_o1 = ps_fx.tile([128, 512], f32, tag="fx")
                av21 = [(fx_e1, 0), (fx_o1, 0)]
                emit_av_mms(2, 1, av21, range(0, 4), True, False)
                emit_release_direct(2, 0, av_cur)
            elif mi == 5:
                emit_av_mms(2, 1, av21, range(4, 8), False, True)
            elif mi == 6:
                emit_release_direct(2, 1, av21)
                av3 = av_pair_tiles()
                emit_av_mms(3, 0, av3, range(0, 2), True, False)
            elif mi == 7:
                emit_av_mms(3, 0, av3, range(2, 6), False, False)
            emit_scores(3, mi)
            if mi == 3:
                # pairs 0-1 reciprocal, batched [4, N] (one slow DVE op),
                # after the pair-2 release copies are already queued.
                rec1 = recp.tile([4, N], bf16, tag="rec")
                nc.vector.reciprocal(out=rec1, in_=gath[0])
                rec1_d = dramp.tile([4, N], bf16, tag="recd")
                nc.sync.dma_start(out=rec1_d, in_=rec1)
                emit_norm_mult(0, rec1_d, 0)
                emit_norm_mult(1, rec1_d, 2)
        # ---- tail ----
        emit_av_mms(3, 0, av3, range(6, 8), False, True)
        emit_release_direct(3, 0, av3)
        fx_e = ps_fx.tile([128, 512], f32, tag="fx")
        fx_o = ps_fx.tile([128, 512], f32, tag="fx")
        av3_1 = [(fx_e, 0), (fx_o, 0)]
        emit_av_mms(3, 1, av3_1, range(8), True, True)
        emit_release_direct(3, 1, av3_1)
        # proj accumulators: all 8 PSUM banks are free (or freeing) now.
        pj = {}
        for oc in range(CCH):
            if oc < 2:
                t = ps_sc.tile([128, N], f32, tag="sc")
            elif oc == 2:
                a2 = ps_fx.tile([128, 512], f32, tag="fx")
                b2 = ps_fx.tile([128, 512], f32, tag="fx")
                pj[(2, 0)], pj[(2, 1)] = a2, b2
                continue
            else:
                t = ps_av.tile([128, N], f32, tag="av")
            pj[(oc, 0)], pj[(oc, 1)] = t[:, 0:512], t[:, 512:1024]

        def emit_proj_cc(ccs, start, stop):
            for oc in range(CCH):
                for nj in range(2):
                    for j, cc in enumerate(ccs):
                        nc.tensor.matmul(
                            pj[(oc, nj)],
                            lhsT=(pw_sb[cc][:, oc * 128 : (oc + 1) * 128]),
                            rhs=(o_sb[cc][:, nj * 512 : (nj + 1) * 512]),
                            start=(start and j == 0),
                            stop=(stop and j == len(ccs) - 1),
                        )

        # Residual rides the accumulation as an identity matmul (x is bf16),
        # so the copy-out is a plain ACT bias-add on the (idle) ACT engine.
        for oc in range(CCH):
            for nj in range(2):
                nc.tensor.matmul(
                    pj[(oc, nj)],
                    lhsT=id_sb,
                    rhs=x_sb[oc][:, nj * 512 : (nj + 1) * 512],
                    start=True,
                    stop=False,
                )
        # partials over the already-normalized pairs 0-1, overlapping the
        # pairs-2/3 reciprocal chain on ACT/DVE
        emit_proj_cc((0, 1), False, False)
        emit_proj_cc((2,), False, False)
        emit_proj_cc((3,), False, True)
        for oc in range(CCH):
            ot = outp.tile([128, N], f32, tag="ot")
            for nj in range(2):
                nc.scalar.activation(
                    out=ot[:, nj * 512 : (nj + 1) * 512],
                    in_=pj[(oc, nj)],
                    func=AF.Identity,
                    bias=pb_sb[oc],
                )
                # issue each half as soon as its copy lands
                nc.sync.dma_start(
                    out=out_d[oc][:, nj * 512 : (nj + 1) * 512],
                    in_=ot[:, nj * 512 : (nj + 1) * 512],
                )

    _add_dram_raw_waits(nc)
    _split_lw_waits(nc)
    return nc


def _add_dram_raw_waits(nc):
    """Tile does not order DMA-after-DMA RAW hazards through DRAM: a DMA
    reading a DRAM scratch tensor gets no wait on the (different-queue) DMA
    that wrote it.  Insert an explicit sem-ge wait on the writer's completion
    semaphore value before each such reader."""
    from concourse import mybir

    for blk in nc.m.functions[0].blocks:
        cum = {}  # semaphore id -> accumulated update value (SP issue order)
        writer = {}  # memsetref name -> (sem id, ant_name, value after write)
        out = []
        for inst in blk.instructions:
            is_dma = type(inst).__name__ == "InstDMACopy"
            si = getattr(inst, "sync_info", None)
            if is_dma:
                for a in inst.ins:
                    n = getattr(a, "memsetref", None)
                    if n in writer:
                        sem_id, ant, val = writer.pop(n)
                        w = mybir.SyncWait(
                            sync_type="semaphore",
                            id=sem_id,
                            ant_name=ant,
                            wait_mode="sem-ge-imm",
                            wait_value=val,
                        )
                        out.append(
                            mybir.InstEventSemaphore(
                                name=f"{inst.name}_dramraw",
                                engine=inst.engine,
                                ins=[],
                                outs=[],
                                sync_info=mybir.SyncInfo(on_wait=[w], on_update=[]),
                            )
                        )
            if si is not None:
                for u in si.on_update or []:
                    if getattr(u, "update_mode", "") == "sem-add-imm":
                        cum[u.id] = cum.get(u.id, 0) + u.update_value
            if is_dma and si is not None:
                for u in si.on_update or []:
                    if getattr(u, "update_mode", "") == "sem-add-imm":
                        for a in inst.outs:
                            n = getattr(a, "memsetref", None)
                            if n:
                                writer[n] = (u.id, u.ant_name, cum[u.id])
            out.append(inst)
        blk.instructions = out


def _split_lw_waits(nc):
    """This walrus build accepts only ONE sync-wait command per engine
    instruction; Tile can attach several. First coalesce sem-ge waits on the
    same semaphore (keep the max threshold), then hoist each excess wait onto
    its own pure sem-wait instruction inserted just before, in queue order."""
    from concourse import mybir

    def coalesce(waits):
        best, rest = {}, []
        for w in waits:
            if (
                getattr(w, "sync_type", "") == "semaphore"
                and getattr(w, "wait_mode", "") == "sem-ge-imm"
            ):
                k = w.id
                if k not in best or best[k].wait_value < w.wait_value:
                    best[k] = w
            else:
                rest.append(w)
        return list(best.values()) + rest

    for blk in nc.m.functions[0].blocks:
        out = []
        for inst in blk.instructions:
            si = getattr(inst, "sync_info", None)
            if si is not None and si.on_wait and len(si.on_wait) > 1:
                inst.sync_info = si = mybir.SyncInfo(
                    on_wait=coalesce(si.on_wait), on_update=list(si.on_update or [])
                )
            if (
                si is not None
                and si.on_wait
                and len(si.on_wait) > 1
                and type(inst).__name__ != "InstEventSemaphore"
            ):
                waits = list(si.on_wait)
                for j, w in enumerate(waits[:-1]):
                    sem = mybir.InstEventSemaphore(
                        name=f"{inst.name}_wsplit{j}",
                        engine=inst.engine,
                        ins=[],
                        outs=[],
                        sync_info=mybir.SyncInfo(on_wait=[w], on_update=[]),
                    )
                    out.append(sem)
                inst.sync_info = mybir.SyncInfo(
                    on_wait=waits[-1:], on_update=list(si.on_update or [])
                )
            out.append(inst)
        blk.instructions = out


def kernel(x, gn_w, gn_b, qkv_w, qkv_b, proj_w, proj_b):
    from concourse.bass_utils import run_bass_kernel_spmd

    B = x.shape[0]
    assert B == NCORES
    if "nc" not in _CACHE:
        _CACHE["nc"] = _build_program()
    nc = _CACHE["nc"]

    scale = float(HD) ** -0.5
    xf = np.ascontiguousarray(
        x.reshape(B, CCH, 128, N).transpose(0, 2, 1, 3).astype(ml_dtypes.bfloat16)
    )
    # layout [128, CCH, 3C]: for input chunk cc the 3C columns are [q | k | v]
    qkv_wT = np.ascontiguousarray(
        np.concatenate(
            [
                qkv_w[0:C].T.reshape(CCH, 128, C),
                (qkv_w[C : 2 * C] * scale).T.reshape(CCH, 128, C),
                qkv_w[2 * C : 3 * C].T.reshape(CCH, 128, C),
            ],
            axis=2,
        )
        .transpose(1, 0, 2)
        .astype(ml_dtypes.bfloat16)
    )
    qb = np.ascontiguousarray(qkv_b[0:C].reshape(CCH, 128).T.astype(np.float32))
    kb = np.ascontiguousarray(
        (qkv_b[C : 2 * C] * scale).reshape(CCH, 128).T.astype(np.float32)
    )
    vb = qkv_b[2 * C : 3 * C]
    pw_T = np.ascontiguousarray(
        proj_w.T.reshape(CCH, 128, C).transpose(1, 0, 2).astype(ml_dtypes.bfloat16)
    )
    pb = np.ascontiguousarray(
        (proj_b + proj_w.astype(np.float64) @ vb.astype(np.float64))
        .astype(np.float32)
        .reshape(CCH, 128, 1)
    )
    gnw = np.ascontiguousarray(gn_w.reshape(CCH, 128).T.astype(np.float32))
    gnb = np.ascontiguousarray(gn_b.reshape(CCH, 128).T.astype(np.float32))
    id128 = np.ascontiguousarray(np.eye(128, dtype=np.float32).astype(ml_dtypes.bfloat16))
    mask = np.zeros((128, 128), dtype=np.float32)
    for g in range(2):
        mask[g * 64 : (g + 1) * 64, g * 64 : (g + 1) * 64] = 1.0 / 64.0

    in_maps = []
    for i in range(NCORES):
        in_maps.append(
            {
                "x": xf[i],
                "qkv_wT": qkv_wT,
                "qb": qb,
                "kb": kb,
                "proj_wT": pw_T,
                "pb": pb,
                "gnw": gnw,
                "gnb": gnb,
                "gn_mask": mask,
                "id128": id128,
            }
        )

    tmpdir = os.environ.get("BASS_TMPDIR")
    if tmpdir:
        os.makedirs(tmpdir, exist_ok=True)
    res = run_bass_kernel_spmd(
        nc,
        in_maps,
        list(range(NCORES)),
        trace=bool(os.environ.get("BASS_TRACE")),
        tmpdir=tmpdir,
    )
    LAST["exec_time_ns"] = res.exec_time_ns
    LAST["results"] = res
    out = np.stack([res.results[i]["out"] for i in range(NCORES)], axis=0)
    return out.reshape(B, C, 32, 32).astype(x.dtype)
